# revision 16
# baseline (speedup 1.0000x reference)
"""Trainium2 Bass kernel for nn_CausalTransformerDecoder (chunked sparse attention).

kernel(**inputs) takes FULL unsharded inputs (B=8, E=512, T=1024), shards the
batch across 8 NeuronCores (1 batch row per core), runs a Bass/Tile kernel,
returns (out [B,E,T], ctx_out [B,NL+1,CTX,E]) matching reference().

Design (per core):
- activations transposed [E-on-partitions (4x128), T-free]; all GEMMs bf16
  with fp32 psum accumulation; residual/LN stream kept fp32.
- projections weight-stationary (produce Q^T/K^T), V via activation-stationary
  (natural [pos, d] layout).
- attention: per 128-q tile, gathered-window scores via overlapping strided
  matmul rhs views. Window mask folded in as a rank-9 staircase matmul into
  the same psum group. Cross-attn / layer-0 positional-encoding handled by
  shifted K-replica tensors (+1 pe matmul) and a pe-add on gathered V.
- softmax: single exp pass (scalar engine) with fused per-head accum, then
  per-partition reciprocal scale; P transposed via xbar DMA-transpose for AV.
- LayerNorm over E (partition dim): ones-matmul partition sums, stats on a
  scattered [128,8] block, per-token rows broadcast back via step-0 DMA.
"""
import numpy as np
import ml_dtypes

import concourse.bass as bass
import concourse.mybir as mybir
import concourse.tile as tile
from concourse import bacc
from concourse.bass_utils import run_bass_kernel_spmd

BF16 = mybir.dt.bfloat16
F32 = mybir.dt.float32
F32R = mybir.dt.float32r
AF = mybir.ActivationFunctionType
OP = mybir.AluOpType

B, E, T = 8, 512, 1024
CTX, CH, NL, NH, FF = 48, 16, 4, 8, 2048
W = CTX + CH          # 64
S = CTX + T           # 1072
HD = E // NH          # 64
EC = E // 128         # 4
FC = FF // 128        # 16
NM = T // 128         # 8 q-tiles
BIG = float(np.float32(1e9).astype(ml_dtypes.bfloat16))


def _view(sliced, free_dims, extra_offset=0):
    """AP with partition dim taken from `sliced` ([P, 1] slice) and custom
    free dims [[elem_step, count], ...]."""
    return bass.AP(tensor=sliced.tensor, offset=sliced.offset + extra_offset,
                   ap=[list(sliced.ap[0])] + [list(d) for d in free_dims])


def _bc_dram(src_1d, nparts, ncols):
    """Partition-broadcast read of a DRAM row [ncols] -> [nparts, ncols]."""
    return bass.AP(tensor=src_1d.tensor, offset=src_1d.offset,
                   ap=[[0, nparts], [1, ncols]])


def build_nc():
    nc = bacc.Bacc("TRN2", target_bir_lowering=False, debug=False, num_devices=8)

    xT0 = nc.dram_tensor("xT0", [E, T], F32, kind="ExternalInput").ap()
    xT0b = nc.dram_tensor("xT0b", [E, T], BF16, kind="ExternalInput").ap()
    memTb = nc.dram_tensor("memTb", [E, S], BF16, kind="ExternalInput").ap()
    ctxTb = nc.dram_tensor("ctxTb", [NL, E, CTX], BF16, kind="ExternalInput").ap()
    saw = nc.dram_tensor("saw", [NL, E, 3 * E], BF16, kind="ExternalInput").ap()
    saow = nc.dram_tensor("saow", [NL, E, E], BF16, kind="ExternalInput").ap()
    caw = nc.dram_tensor("caw", [NL, E, 3 * E], BF16, kind="ExternalInput").ap()
    caow = nc.dram_tensor("caow", [NL, E, E], BF16, kind="ExternalInput").ap()
    l1w = nc.dram_tensor("l1w", [NL, E, FF], BF16, kind="ExternalInput").ap()
    l2w = nc.dram_tensor("l2w", [NL, FF, E], BF16, kind="ExternalInput").ap()
    scal = nc.dram_tensor("scal", [NL, 128, 76], F32, kind="ExternalInput").ap()
    vrow = nc.dram_tensor("vrow", [NL, 2, E], BF16, kind="ExternalInput").ap()
    um8 = nc.dram_tensor("um8", [9, 128], BF16, kind="ExternalInput").ap()
    vm8 = nc.dram_tensor("vm8", [9, 512], BF16, kind="ExternalInput").ap()
    vm2 = nc.dram_tensor("vm2", [9, 256], BF16, kind="ExternalInput").ap()
    pekq = nc.dram_tensor("pekq", [E, CH], BF16, kind="ExternalInput").ap()
    peres = nc.dram_tensor("peres", [E, CH], F32, kind="ExternalInput").ap()
    pek = nc.dram_tensor("pek", [NL + 1, E, W], BF16, kind="ExternalInput").ap()
    pev2 = nc.dram_tensor("pev2", [NL + 1, 128, E], BF16, kind="ExternalInput").ap()

    outT = nc.dram_tensor("outT", [E, T], F32, kind="ExternalOutput").ap()
    ctxo = nc.dram_tensor("ctxo", [NL - 1, E, CTX], F32, kind="ExternalOutput").ap()
    lnrow = nc.dram_tensor("lnrow", [NL, 3, 2, T], F32).ap()

    with tile.TileContext(nc) as tc, \
         tc.tile_pool(name="per", bufs=1) as per, \
         tc.tile_pool(name="wts", bufs=1) as wts, \
         tc.tile_pool(name="act", bufs=1) as actp, \
         tc.tile_pool(name="kv", bufs=1) as kvp, \
         tc.tile_pool(name="krep", bufs=1) as krp, \
         tc.tile_pool(name="attn", bufs=2) as atp, \
         tc.tile_pool(name="lnp", bufs=1) as lnp, \
         tc.tile_pool(name="scps", bufs=2, space="PSUM") as scps, \
         tc.tile_pool(name="avps", bufs=2, space="PSUM") as avps, \
         tc.tile_pool(name="prps", bufs=2, space="PSUM") as prps:

        ones_b = per.tile([1, 128], BF16)
        nc.vector.memset(ones_b[:], 1.0)
        ones_f = per.tile([128, 1], F32)
        nc.vector.memset(ones_f[:], 1.0)
        eps_c = per.tile([128, 1], F32)
        nc.vector.memset(eps_c[:], 1e-5)
        um_sb = per.tile([9, 128], BF16)
        nc.sync.dma_start(out=um_sb[:], in_=um8[:])
        vm8_sb = per.tile([9, 512], BF16)
        nc.sync.dma_start(out=vm8_sb[:], in_=vm8[:])
        vm2_sb = per.tile([9, 256], BF16)
        nc.sync.dma_start(out=vm2_sb[:], in_=vm2[:])
        pekq_sb = per.tile([128, EC, CH], BF16)
        nc.sync.dma_start(out=pekq_sb[:], in_=pekq.rearrange("(c p) j -> p c j", p=128))
        peres_sb = per.tile([128, EC, CH], F32)
        nc.sync.dma_start(out=peres_sb[:], in_=peres.rearrange("(c p) j -> p c j", p=128))
        memT_sb = per.tile([128, EC, S], BF16)
        nc.sync.dma_start(out=memT_sb[:], in_=memTb.rearrange("(c p) s -> p c s", p=128))

        x = actp.tile([128, EC, T], F32, tag="x")
        xb = actp.tile([128, EC, T], BF16, tag="xb")
        for c in range(EC):
            nc.sync.dma_start(out=x[:, c, :], in_=xT0[128 * c:128 * c + 128, :])
            nc.sync.dma_start(out=xb[:, c, :], in_=xT0b[128 * c:128 * c + 128, :])

        def _vrows(vsrc, vg, kk, prow, start, n, csl):
            """V rows [start, start+n) (global pos), cols csl -> vg[prow:+n, kk, :].
            vsrc: list of (tile, global_start, length); each tile holds its rows
            as [partition r%128, block r//128, E]."""
            done = 0
            while done < n:
                r = start + done
                for (tl, g0, ln) in vsrc:
                    if g0 <= r < g0 + ln:
                        loc = r - g0
                        p0, blk = loc % 128, loc // 128
                        take = min(n - done, 128 - p0, g0 + ln - r)
                        nc.sync.dma_start(
                            out=vg[prow + done:prow + done + take, kk, :],
                            in_=tl[p0:p0 + take, blk, csl])
                        done += take
                        break
                else:
                    raise ValueError("bad vrow range")

        def emit_attention(QTb, KTb, vsrc, vm_sb, pek_sb, pev2_sb, OTb, g8):
            """g8 True: 512 gathered cols (+pe); False: 256 quad cols, no pe."""
            KK = 4 if g8 else 2
            N = 128 * KK
            has_pe = g8 and pek_sb is not None
            for hp in range(4):
                if has_pe:
                    kr = krp.tile([128, 4, 1088], BF16, tag="krep")
                    for s in range(4):
                        ln_s = S - 16 * s
                        pe_in1 = _view(pek_sb[:, hp, 0:1],
                                       [[0, ln_s // 16], [1, 16]],
                                       extra_offset=16 * s)
                        nc.vector.scalar_tensor_tensor(
                            out=kr[:, s, 0:ln_s],
                            in0=KTb[:, hp, 16 * s:16 * s + ln_s],
                            scalar=1.0, in1=pe_in1,
                            op0=OP.mult, op1=OP.add)
                for m in range(NM):
                    # gather this head-pair's V columns for the tile's windows
                    vg = atp.tile([128, KK, 128], BF16, tag="vg")
                    csl = slice(128 * hp, 128 * hp + 128)
                    for kk in range(KK):
                        if g8:
                            _vrows(vsrc, vg, kk, 0, 128 * m + 32 * kk, 64, csl)
                            _vrows(vsrc, vg, kk, 64, 128 * m + 32 * kk + 16, 64, csl)
                        else:
                            _vrows(vsrc, vg, kk, 0, 128 * m + 64 * kk, 128, csl)
                    if pev2_sb is not None:
                        for kk in range(KK):
                            nc.any.tensor_add(vg[:, kk, :], vg[:, kk, :],
                                              pev2_sb[:, 128 * hp:128 * hp + 128])
                    sc = scps.tile([128, 2, N], F32, tag="scps")
                    expv = atp.tile([128, 2, N], BF16, tag="expv")
                    ssum = atp.tile([128, 2], F32, tag="ssum")
                    for hh in range(2):
                        qap = QTb[64 * hh:64 * hh + 64, hp, 128 * m:128 * m + 128]
                        nc.tensor.matmul(sc[:, hh, :], lhsT=um_sb[:],
                                         rhs=vm_sb[:, 0:N], start=True, stop=False)
                        if has_pe:
                            ks = kr[64 * hh:64 * hh + 64, 0, 0:1]
                            kview = _view(ks, [[16, 8], [1088, 4], [1, 16]],
                                          extra_offset=128 * m)
                        else:
                            ks = KTb[64 * hh:64 * hh + 64, hp, 0:1]
                            kview = _view(ks, [[64, 2], [1, 128]],
                                          extra_offset=128 * m)
                        nc.tensor.matmul(sc[:, hh, :], lhsT=qap, rhs=kview,
                                         start=False, stop=not has_pe)
                        if has_pe:
                            psrc = pek_sb[64 * hh:64 * hh + 64, hp, 0:1]
                            pview = _view(psrc, [[0, 8], [1, 64]])
                            nc.tensor.matmul(sc[:, hh, :], lhsT=qap, rhs=pview,
                                             start=False, stop=True)
                        nc.scalar.activation(expv[:, hh, :], sc[:, hh, :], AF.Exp,
                                             accum_out=ssum[:, hh:hh + 1])
                    rec = atp.tile([128, 2], F32, tag="rec")
                    nc.vector.reciprocal(rec[:], ssum[:])
                    for hh in range(2):
                        nc.any.tensor_scalar_mul(expv[:, hh, :], expv[:, hh, :],
                                                 rec[:, hh:hh + 1])
                    av = avps.tile([128, 128], F32, tag="avps")
                    for hh in range(2):
                        ext = atp.tile([128, KK, 128], BF16, tag="expT")
                        for kk in range(KK):
                            nc.sync.dma_start_transpose(
                                out=ext[:, kk, :],
                                in_=expv[:, hh, 128 * kk:128 * kk + 128])
                        for kk in range(KK):
                            nc.tensor.matmul(
                                av[64 * hh:64 * hh + 64, :],
                                lhsT=vg[:, kk, 64 * hh:64 * hh + 64],
                                rhs=ext[:, kk, :],
                                start=(kk == 0), stop=(kk == KK - 1))
                    nc.any.tensor_copy(OTb[:, hp, 128 * m:128 * m + 128], av[:])

        def emit_ln(i, which, y_tiles, dst_f32, dst_b16, g_col, b_col):
            sums_sb = lnp.tile([128, 8], F32, tag="lnsums")
            sq_sb = lnp.tile([128, 8], F32, tag="lnsq")
            for t in range(2):
                yt = y_tiles[t]
                ysq = lnp.tile([128, EC, 512], F32, tag="ysq")
                nc.any.tensor_mul(ysq[:], yt[:], yt[:])
                ps_s = prps.tile([1, 512], F32, tag="proj")
                ps_q = prps.tile([1, 512], F32, tag="proj")
                for c in range(EC):
                    nc.tensor.matmul(ps_s[:], lhsT=ones_f[:], rhs=yt[:, c, :],
                                     start=(c == 0), stop=(c == EC - 1))
                for c in range(EC):
                    nc.tensor.matmul(ps_q[:], lhsT=ones_f[:], rhs=ysq[:, c, :],
                                     start=(c == 0), stop=(c == EC - 1))
                row_s = lnp.tile([1, 512], F32, tag="rows")
                row_q = lnp.tile([1, 512], F32, tag="rowq")
                nc.any.tensor_copy(row_s[:], ps_s[:])
                nc.any.tensor_copy(row_q[:], ps_q[:])
                nc.sync.dma_start(out=sums_sb[:, 4 * t:4 * t + 4], in_=row_s[:])
                nc.sync.dma_start(out=sq_sb[:, 4 * t:4 * t + 4], in_=row_q[:])
            mq = lnp.tile([128, 8], F32, tag="lnmq")
            nc.vector.tensor_scalar_mul(mq[:], sums_sb[:], 1.0 / E)
            vq = lnp.tile([128, 8], F32, tag="lnvq")
            nc.vector.tensor_scalar_mul(vq[:], sq_sb[:], 1.0 / E)
            msq = lnp.tile([128, 8], F32, tag="lnmsq")
            nc.vector.tensor_mul(msq[:], mq[:], mq[:])
            nc.vector.tensor_sub(vq[:], vq[:], msq[:])
            rstd = lnp.tile([128, 8], F32, tag="lnrstd")
            nc.scalar.activation(rstd[:], vq[:], AF.Sqrt, bias=eps_c[:])
            nc.vector.reciprocal(rstd[:], rstd[:])
            mr = lnp.tile([128, 8], F32, tag="lnmr")
            nc.vector.tensor_mul(mr[:], mq[:], rstd[:])
            for t in range(2):
                nc.sync.dma_start(
                    out=lnrow[i, which, 0, 512 * t:512 * t + 512].unsqueeze(0),
                    in_=rstd[:, 4 * t:4 * t + 4])
                nc.sync.dma_start(
                    out=lnrow[i, which, 1, 512 * t:512 * t + 512].unsqueeze(0),
                    in_=mr[:, 4 * t:4 * t + 4])
            for t in range(2):
                rbc = lnp.tile([128, 512], F32, tag="rbc")
                mbc = lnp.tile([128, 512], F32, tag="mbc")
                nc.sync.dma_start(
                    out=rbc[:], in_=_bc_dram(lnrow[i, which, 0, 512 * t:512 * t + 512],
                                             128, 512))
                nc.sync.dma_start(
                    out=mbc[:], in_=_bc_dram(lnrow[i, which, 1, 512 * t:512 * t + 512],
                                             128, 512))
                for c in range(EC):
                    t1 = lnp.tile([128, 512], F32, tag="lnt1")
                    nc.any.tensor_mul(t1[:], y_tiles[t][:, c, :], rbc[:])
                    nc.any.tensor_sub(t1[:], t1[:], mbc[:])
                    nc.vector.scalar_tensor_tensor(
                        out=dst_f32[:, c, 512 * t:512 * t + 512], in0=t1[:],
                        scalar=g_col[:, c:c + 1],
                        in1=b_col[:, c:c + 1].to_broadcast((128, 512)),
                        op0=OP.mult, op1=OP.add)
                    nc.any.tensor_copy(dst_b16[:, c, 512 * t:512 * t + 512],
                                       dst_f32[:, c, 512 * t:512 * t + 512])

        def proj_to(dst, wt_sb, col0, src_b16, bias_col, scale=1.0, act=AF.Identity):
            """dst[:, co, 512t:+512] = act(src @ W-block + bias), over T cols."""
            n_out = dst.shape[1]
            for co in range(n_out):
                for t in range(2):
                    ps = prps.tile([128, 512], F32, tag="proj")
                    for k in range(EC):
                        nc.tensor.matmul(
                            ps[:], lhsT=wt_sb[:, k, col0 + 128 * co:col0 + 128 * co + 128],
                            rhs=src_b16[:, k, 512 * t:512 * t + 512],
                            start=(k == 0), stop=(k == EC - 1))
                    nc.scalar.activation(dst[:, co, 512 * t:512 * t + 512], ps[:],
                                         act, bias=bias_col[:, co:co + 1], scale=scale)

        def proj_res(y_tiles, wt_sb, src_b16, ob_col, res, n_k=EC):
            for co in range(EC):
                for t in range(2):
                    ps = prps.tile([128, 512], F32, tag="proj")
                    for k in range(n_k):
                        nc.tensor.matmul(
                            ps[:], lhsT=wt_sb[:, k, 128 * co:128 * co + 128],
                            rhs=src_b16[:, k, 512 * t:512 * t + 512],
                            start=(k == 0), stop=(k == n_k - 1))
                    nc.vector.scalar_tensor_tensor(
                        out=y_tiles[t][:, co, :], in0=ps[:],
                        scalar=ob_col[:, co:co + 1],
                        in1=res[:, co, 512 * t:512 * t + 512],
                        op0=OP.add, op1=OP.add)

        for i in range(NL):
            sc_sb = wts.tile([128, 76], F32, tag="scal")
            nc.sync.dma_start(out=sc_sb[:], in_=scal[i])
            saw_sb = wts.tile([128, EC, 3 * E], BF16, tag="bigw", bufs=2)
            nc.sync.dma_start(out=saw_sb[:], in_=saw[i].rearrange("(c p) o -> p c o", p=128))
            saow_sb = wts.tile([128, EC, E], BF16, tag="ow", bufs=2)
            nc.sync.dma_start(out=saow_sb[:], in_=saow[i].rearrange("(c p) o -> p c o", p=128))
            ctx_sb = wts.tile([128, EC, CTX], BF16, tag="ctx")
            nc.sync.dma_start(out=ctx_sb[:], in_=ctxTb[i].rearrange("(c p) o -> p c o", p=128))
            vrow_sa = wts.tile([1, E], BF16, tag="vrowsa")
            nc.sync.dma_start(out=vrow_sa[:], in_=vrow[i, 0:1, :])
            vrow_ca = wts.tile([1, E], BF16, tag="vrowca")
            nc.sync.dma_start(out=vrow_ca[:], in_=vrow[i, 1:2, :])
            if i == 0:
                pek_sa = wts.tile([128, EC, W], BF16, tag="peksa")
                nc.sync.dma_start(out=pek_sa[:],
                                  in_=pek[0].rearrange("(c p) j -> p c j", p=128))
                pev2_sa = wts.tile([128, E], BF16, tag="pev2sa")
                nc.sync.dma_start(out=pev2_sa[:], in_=pev2[0])
            else:
                pek_sa = pev2_sa = None
            pek_ca = wts.tile([128, EC, W], BF16, tag="pekca")
            nc.sync.dma_start(out=pek_ca[:],
                              in_=pek[i + 1].rearrange("(c p) j -> p c j", p=128))
            pev2_ca = wts.tile([128, E], BF16, tag="pev2ca")
            nc.sync.dma_start(out=pev2_ca[:], in_=pev2[i + 1])

            if i >= 1:
                for c in range(EC):
                    nc.sync.dma_start(out=ctxo[i - 1, 128 * c:128 * c + 128, :],
                                      in_=x[:, c, T - CTX:T])

            # ---------- self-attention ----------
            QTb = kvp.tile([128, EC, T], BF16, tag="qh")
            proj_to(QTb, saw_sb, 0, xb, sc_sb[:, 0:4], scale=0.125)
            if i == 0:
                for c in range(EC):
                    nc.any.tensor_add(
                        QTb[:, c, :], QTb[:, c, :],
                        _view(pekq_sb[:, c, 0:1], [[0, T // CH], [1, CH]]))

            KTb = kvp.tile([128, EC, S + 16], BF16, tag="KTb")
            nc.vector.memset(KTb[:, :, S:S + 16], 0.0)
            for co in range(EC):
                pos = 0
                for ncol, src in ((CTX, None), (512, 0), (512, 512)):
                    ps = prps.tile([128, 512], F32, tag="proj")
                    for k in range(EC):
                        rhs = ctx_sb[:, k, :] if src is None else xb[:, k, src:src + ncol]
                        nc.tensor.matmul(
                            ps[:, 0:ncol],
                            lhsT=saw_sb[:, k, E + 128 * co:E + 128 * co + 128],
                            rhs=rhs, start=(k == 0), stop=(k == EC - 1))
                    nc.scalar.activation(KTb[:, co, pos:pos + ncol], ps[:, 0:ncol],
                                         AF.Identity, bias=sc_sb[:, 4 + co:5 + co])
                    pos += ncol

            Vc = kvp.tile([48, 1, E], BF16, tag="Vc")
            Vx = kvp.tile([128, 9, E], BF16, tag="Vnat")
            nc.vector.memset(Vx[0:16, 8, :], 0.0)
            for p in range(9):
                nr = CTX if p == 8 else 128
                ps = prps.tile([128, 512], F32, tag="proj")
                for k in range(EC):
                    lhs = (ctx_sb[:, k, 0:CTX] if p == 8
                           else xb[:, k, 128 * p:128 * p + 128])
                    nc.tensor.matmul(ps[0:nr, :], lhsT=lhs,
                                     rhs=saw_sb[:, k, 2 * E:3 * E],
                                     start=(k == 0), stop=False)
                nc.tensor.matmul(ps[0:nr, :], lhsT=ones_b[0:1, 0:nr], rhs=vrow_sa[:],
                                 start=False, stop=True)
                if p == 8:
                    nc.any.tensor_copy(Vc[:, 0, :], ps[0:CTX, :])
                else:
                    nc.any.tensor_copy(Vx[:, p, :], ps[:])

            OTb = kvp.tile([128, EC, T], BF16, tag="OTb")
            emit_attention(QTb, KTb, [(Vc, 0, CTX), (Vx, CTX, T + 16)],
                           vm8_sb if i == 0 else vm2_sb,
                           pek_sa, pev2_sa, OTb, g8=(i == 0))

            y1 = [actp.tile([128, EC, 512], F32, tag=f"y_t{t}", name=f"y1_{i}_{t}") for t in range(2)]
            proj_res(y1, saow_sb, OTb, sc_sb[:, 12:16], x)
            if i == 0:
                # layer-0 residual includes the positional encoding (q = tc[:, -CH:])
                for t in range(2):
                    for c in range(EC):
                        nc.any.tensor_add(
                            y1[t][:, c, :], y1[t][:, c, :],
                            _view(peres_sb[:, c, 0:1], [[0, 512 // CH], [1, CH]]))
            x1 = actp.tile([128, EC, T], F32, tag="x1")
            x1b = actp.tile([128, EC, T], BF16, tag="x1b")
            emit_ln(i, 0, y1, x1, x1b, sc_sb[:, 52:56], sc_sb[:, 56:60])

            # ---------- cross-attention ----------
            caw_sb = wts.tile([128, EC, 3 * E], BF16, tag="bigw", bufs=2)
            nc.sync.dma_start(out=caw_sb[:], in_=caw[i].rearrange("(c p) o -> p c o", p=128))
            caow_sb = wts.tile([128, EC, E], BF16, tag="ow", bufs=2)
            nc.sync.dma_start(out=caow_sb[:], in_=caow[i].rearrange("(c p) o -> p c o", p=128))

            QTc = kvp.tile([128, EC, T], BF16, tag="qh")
            proj_to(QTc, caw_sb, 0, x1b, sc_sb[:, 16:20], scale=0.125)
            KTc = kvp.tile([128, EC, S + 16], BF16, tag="KTb")
            nc.vector.memset(KTc[:, :, S:S + 16], 0.0)
            for co in range(EC):
                pos = 0
                for ncol in (512, 512, 48):
                    ps = prps.tile([128, 512], F32, tag="proj")
                    for k in range(EC):
                        nc.tensor.matmul(
                            ps[:, 0:ncol],
                            lhsT=caw_sb[:, k, E + 128 * co:E + 128 * co + 128],
                            rhs=memT_sb[:, k, pos:pos + ncol],
                            start=(k == 0), stop=(k == EC - 1))
                    nc.scalar.activation(KTc[:, co, pos:pos + ncol], ps[:, 0:ncol],
                                         AF.Identity, bias=sc_sb[:, 20 + co:21 + co])
                    pos += ncol
            Vnc = kvp.tile([128, 9, E], BF16, tag="Vnat")
            for p in range(9):
                nr = 128 if p < 8 else 48
                ps = prps.tile([128, 512], F32, tag="proj")
                for k in range(EC):
                    nc.tensor.matmul(ps[0:nr, :],
                                     lhsT=memT_sb[:, k, 128 * p:128 * p + nr],
                                     rhs=caw_sb[:, k, 2 * E:3 * E],
                                     start=(k == 0), stop=False)
                nc.tensor.matmul(ps[0:nr, :], lhsT=ones_b[0:1, 0:nr], rhs=vrow_ca[:],
                                 start=False, stop=True)
                nc.any.tensor_copy(Vnc[0:nr, p, :], ps[0:nr, :])

            OTc = kvp.tile([128, EC, T], BF16, tag="OTb")
            emit_attention(QTc, KTc, [(Vnc, 0, S)], vm8_sb, pek_ca, pev2_ca,
                           OTc, g8=True)

            y2 = [actp.tile([128, EC, 512], F32, tag=f"y_t{t}", name=f"y2_{i}_{t}") for t in range(2)]
            proj_res(y2, caow_sb, OTc, sc_sb[:, 28:32], x1)
            x2 = actp.tile([128, EC, T], F32, tag="x1")
            x2b = actp.tile([128, EC, T], BF16, tag="x1b")
            emit_ln(i, 1, y2, x2, x2b, sc_sb[:, 60:64], sc_sb[:, 64:68])

            # ---------- feed-forward ----------
            l1_sb = wts.tile([128, EC, FF], BF16, tag="bigw", bufs=2)
            nc.sync.dma_start(out=l1_sb[:], in_=l1w[i].rearrange("(c p) o -> p c o", p=128))
            l2_sb = wts.tile([128, FC, E], BF16, tag="bigw", bufs=2)
            nc.sync.dma_start(out=l2_sb[:], in_=l2w[i].rearrange("(c p) o -> p c o", p=128))
            y3 = [actp.tile([128, EC, 512], F32, tag=f"y_t{t}", name=f"y3_{i}_{t}") for t in range(2)]
            for t in range(2):
                hb = kvp.tile([128, FC, 512], BF16, tag="qh", name=f"hb_{i}_{t}")
                for f in range(FC):
                    ps = prps.tile([128, 512], F32, tag="proj")
                    for k in range(EC):
                        nc.tensor.matmul(ps[:], lhsT=l1_sb[:, k, 128 * f:128 * f + 128],
                                         rhs=x2b[:, k, 512 * t:512 * t + 512],
                                         start=(k == 0), stop=(k == EC - 1))
                    nc.scalar.activation(hb[:, f, :], ps[:], AF.Relu,
                                         bias=sc_sb[:, 32 + f:33 + f])
                for co in range(EC):
                    ps = prps.tile([128, 512], F32, tag="proj")
                    for k in range(FC):
                        nc.tensor.matmul(ps[:], lhsT=l2_sb[:, k, 128 * co:128 * co + 128],
                                         rhs=hb[:, k, :],
                                         start=(k == 0), stop=(k == FC - 1))
                    nc.vector.scalar_tensor_tensor(
                        out=y3[t][:, co, :], in0=ps[:], scalar=sc_sb[:, 48 + co:49 + co],
                        in1=x2[:, co, 512 * t:512 * t + 512],
                        op0=OP.add, op1=OP.add)
            emit_ln(i, 2, y3, x, xb, sc_sb[:, 68:72], sc_sb[:, 72:76])

        for c in range(EC):
            nc.sync.dma_start(out=outT[128 * c:128 * c + 128, :], in_=x[:, c, :])

    nc.compile()
    return nc


_NC_CACHE = {}


def _get_nc():
    if "nc" not in _NC_CACHE:
        _NC_CACHE["nc"] = build_nc()
    return _NC_CACHE["nc"]


def _pack_cols(*vecs):
    cols = [np.asarray(v, np.float32).reshape(-1, 128).T for v in vecs]
    return np.concatenate(cols, axis=1)


def _masks():
    um = np.zeros((9, 128), np.float32)
    vm8v = np.zeros((9, 512), np.float32)
    vm2v = np.zeros((9, 256), np.float32)
    um[0, :] = 1.0
    vm8v[0, :] = -BIG
    vm2v[0, :] = -BIG
    for g in range(8):
        um[1 + g, 16 * g:16 * g + 16] = 1.0
        vm8v[1 + g, 64 * g:64 * g + 64] = BIG
        q, r = divmod(g, 4)
        vm2v[1 + g, 128 * q + 16 * r:128 * q + 16 * r + 64] = BIG
    bf = ml_dtypes.bfloat16
    return um.astype(bf), vm8v.astype(bf), vm2v.astype(bf)


def prepare_in_maps(tgt, mem, ctx_buf, pe, sa_w, sa_b, sa_ow, sa_ob, ca_w, ca_b,
                    ca_ow, ca_ob, l1_w, l1_b, l2_w, l2_b, n1_g, n1_b, n2_g, n2_b,
                    n3_g, n3_b):
    bf = ml_dtypes.bfloat16
    f32 = np.float32
    tgt = np.asarray(tgt, f32)
    mem = np.asarray(mem, f32)
    ctx_buf = np.asarray(ctx_buf, f32)
    pe = np.asarray(pe, f32)
    sa_w, sa_b, sa_ow, sa_ob = (np.asarray(a, f32) for a in (sa_w, sa_b, sa_ow, sa_ob))
    ca_w, ca_b, ca_ow, ca_ob = (np.asarray(a, f32) for a in (ca_w, ca_b, ca_ow, ca_ob))
    l1_w, l1_b, l2_w, l2_b = (np.asarray(a, f32) for a in (l1_w, l1_b, l2_w, l2_b))
    n1_g, n1_b = np.asarray(n1_g, f32), np.asarray(n1_b, f32)
    n2_g, n2_b = np.asarray(n2_g, f32), np.asarray(n2_b, f32)
    n3_g, n3_b = np.asarray(n3_g, f32), np.asarray(n3_b, f32)

    um, vm8v, vm2v = _masks()

    shared = {
        "saw": sa_w.transpose(0, 2, 1).astype(bf).copy(),
        "saow": sa_ow.transpose(0, 2, 1).astype(bf).copy(),
        "caw": ca_w.transpose(0, 2, 1).astype(bf).copy(),
        "caow": ca_ow.transpose(0, 2, 1).astype(bf).copy(),
        "l1w": l1_w.transpose(0, 2, 1).astype(bf).copy(),
        "l2w": l2_w.transpose(0, 2, 1).astype(bf).copy(),
        "um8": um, "vm8": vm8v, "vm2": vm2v,
        "pekq": ((pe[CTX:W] @ sa_w[0][:E].T) * 0.125).T.astype(bf).copy(),
        "peres": pe[CTX:W].T.astype(f32).copy(),
    }
    shared["scal"] = np.stack([
        _pack_cols(sa_b[i][:E] * 0.125, sa_b[i][E:2 * E], sa_b[i][2 * E:], sa_ob[i],
                   ca_b[i][:E] * 0.125, ca_b[i][E:2 * E], ca_b[i][2 * E:], ca_ob[i],
                   l1_b[i], l2_b[i], n1_g[i], n1_b[i], n2_g[i], n2_b[i],
                   n3_g[i], n3_b[i])
        for i in range(NL)])
    shared["vrow"] = np.stack([np.stack([sa_b[i][2 * E:], ca_b[i][2 * E:]])
                               for i in range(NL)]).astype(bf)
    peks = [pe[:W] @ sa_w[0][E:2 * E].T] + [pe[:W] @ ca_w[i][E:2 * E].T
                                            for i in range(NL)]
    pevs = [pe[:W] @ sa_w[0][2 * E:].T] + [pe[:W] @ ca_w[i][2 * E:].T
                                           for i in range(NL)]
    shared["pek"] = np.stack([p.T for p in peks]).astype(bf).copy()
    shared["pev2"] = np.stack([np.concatenate([p, p], 0) for p in pevs]).astype(bf).copy()

    in_maps = []
    for b in range(B):
        m = dict(shared)
        m["xT0"] = tgt[b].copy()
        m["xT0b"] = tgt[b].astype(bf)
        memfull = np.concatenate([ctx_buf[b, 0].T, mem[b]], axis=1)
        m["memTb"] = memfull.astype(bf)
        m["ctxTb"] = ctx_buf[b, 1:].transpose(0, 2, 1).astype(bf).copy()
        in_maps.append(m)
    return in_maps


def kernel(tgt, mem, ctx_buf, pe, **kw):
    f32 = np.float32
    tgt = np.asarray(tgt, f32)
    mem = np.asarray(mem, f32)
    ctx_buf = np.asarray(ctx_buf, f32)
    in_maps = prepare_in_maps(tgt, mem, ctx_buf, pe, **kw)
    nc = _get_nc()
    res = run_bass_kernel_spmd(nc, in_maps, list(range(B)))
    _NC_CACHE["last_result"] = res

    out = np.stack([np.asarray(res.results[b]["outT"], f32) for b in range(B)])
    ctx_out = np.zeros((B, NL + 1, CTX, E), f32)
    for b in range(B):
        memfull = np.concatenate([ctx_buf[b, 0].T, mem[b]], axis=1)
        ctx_out[b, 0] = memfull[:, -CTX:].T
        ctx_out[b, 1] = tgt[b][:, -CTX:].T
        co = np.asarray(res.results[b]["ctxo"], f32)
        for j in range(NL - 1):
            ctx_out[b, 2 + j] = co[j].T
    return out, ctx_out
